# revision 1
# baseline (speedup 1.0000x reference)
"""Trainium2 Bass kernel for batched dot-product attention scores + softmax.

hidden: [1, 32, 1024] f32, encoder_outputs: [4096, 32, 1024] f32
out[b, 0, l] = softmax_l( sum_h hidden[0,b,h] * encoder_outputs[l,b,h] )

Sharding: batch dim (32) split 4-per-core across 8 NeuronCores (pure data
parallel). Each core streams its 64 MiB encoder_outputs shard once.

Per-core plan (B=4 local batches, L=4096, H=1024, P=128 partitions):
  - hidden shard broadcast to all 128 partitions once: hb [128, 4, 1024].
  - Batch-major streaming: per batch, 8 DMAs of 2 MiB (4 l-blocks each,
    4 KiB contiguous runs), each l-block consumed by a fused DVE
    scalar_tensor_tensor pass (mul + row-sum in one instruction). The DVE
    stream (~147us) hides under the ~190us/core HBM roofline, and each
    batch's softmax chain overlaps the next batch's DMA stream — only the
    last batch's chain sits in the kernel tail.
  - Softmax with partition-local stats + gpsimd partition_all_reduce for
    the cross-partition max/sum (flash-style rescale, exact in exact
    arithmetic).
  - DVE 32x32 stream-transposes so the store to HBM has contiguous runs.
"""

import numpy as np


def _ensure_concourse():
    try:
        import concourse.bass  # noqa: F401
    except ModuleNotFoundError:
        import sys

        for p in ("/opt/trn_rl_repo", "/root/.axon_site/_ro/trn_rl_repo"):
            if p not in sys.path:
                sys.path.insert(0, p)
        import concourse.bass  # noqa: F401


L = 4096
B_TOTAL = 32
H = 1024
N_CORES = 8
B = B_TOTAL // N_CORES  # 4 local batches per core
P = 128
NT = L // P  # 32 l-tiles

_CACHE = {}


def _body(tc, e_ap, h_ap, o_ap, reps=1):
    import concourse.bass as bass
    from concourse import mybir, bass_isa

    nc = tc.nc
    f32 = mybir.dt.float32
    Alu = mybir.AluOpType
    Act = mybir.ActivationFunctionType

    # [4096, 4, 1024] -> [32, 128, 4, 1024]
    e_r = e_ap.rearrange("(n p) b h -> n p b h", p=P)

    with (
        tc.tile_pool(name="consts", bufs=1) as consts,
        tc.tile_pool(name="epool", bufs=6) as epool,
        tc.tile_pool(name="scratch", bufs=1) as scratch,
        tc.tile_pool(name="small", bufs=2) as small,
    ):
        # hidden shard replicated across all 128 partitions; one DMA per
        # batch so the first STT only waits for its own batch's row.
        hb = consts.tile([P, B, H], f32)
        for b in range(B):
            h_row = h_ap[b : b + 1, :]
            h_bcast = bass.AP(
                tensor=h_row.tensor,
                offset=h_row.offset,
                ap=[[0, P]] + [list(h_row.ap[-1])],
            )
            nc.gpsimd.dma_start(out=hb[:, b, :], in_=h_bcast)

        # Warm the ACT Exp spline table while the kernel is DMA-bound so the
        # softmax tail doesn't pay the ~2.7us table load.
        warm = consts.tile([P, 1], f32)
        nc.vector.memset(warm[:], 0.0)
        nc.scalar.activation(out=warm[:], in_=warm[:], func=Act.Exp)

        for _rep in range(reps):
            _rep_body(tc, e_ap, o_ap, hb, epool, scratch, small)


def _rep_body(tc, e_ap, o_ap, hb, epool, scratch, small):
    import concourse.bass as bass
    from concourse import mybir, bass_isa

    nc = tc.nc
    f32 = mybir.dt.float32
    Alu = mybir.AluOpType
    Act = mybir.ActivationFunctionType
    KB = 4  # l-blocks per DMA tile (4 x 512 KiB = 2 MiB)

    o_r = o_ap.rearrange("b (c j p) -> b j c p", c=32, j=P // 32, p=32)

    # Batch-major streaming: all of batch b's tiles before batch b+1, so each
    # batch's softmax chain overlaps the next batch's DMA stream and only the
    # last batch's chain sits in the kernel tail.
    for b in range(B):
        scores = small.tile([P, NT], f32, tag="scores")
        prod = scratch.tile([P, H], f32, tag="prod")
        for t in range(NT // KB):
            et = epool.tile([P, KB, H], f32, tag="et")
            # KB l-blocks of batch b in one 2 MiB DMA (4 KiB contiguous runs)
            src = bass.AP(
                tensor=e_ap.tensor,
                offset=t * KB * P * B * H + b * H,
                ap=[
                    [B * H, P],       # l within block (16 KiB stride)
                    [P * B * H, KB],  # l-block (2 MiB stride)
                    [1, H],           # h contiguous
                ],
            )
            if (t == 0 and b == 0) or (t == NT // KB - 1 and b == B - 1):
                # Split the first tile (first STT starts after 512 KiB, not
                # 2 MiB) and the last tile (final STTs pipeline with the
                # arriving chunks instead of waiting for the full 2 MiB,
                # pulling the tail softmax chain ~3.5us earlier).
                for k in range(KB):
                    nc.sync.dma_start(out=et[:, k, :], in_=src[:, k, :])
            else:
                nc.sync.dma_start(out=et[:], in_=src)
            for k in range(KB):
                i = t * KB + k
                # out = (et * 1.0) * hb, accum_out = sum — one fused DVE pass
                # (tensor_tensor_reduce opcode is rejected by this runtime).
                nc.vector.scalar_tensor_tensor(
                    out=prod[:],
                    in0=et[:, k, :],
                    scalar=1.0,
                    in1=hb[:, b, :],
                    op0=Alu.mult,
                    op1=Alu.mult,
                    accum_out=scores[:, i : i + 1],
                )

        # ---- softmax for batch b (overlaps batch b+1's stream) ----
        # scores[p, i] holds score at l = 128*i + p.
        mst = small.tile([P, 1], f32, tag="mst")
        negm = small.tile([P, 1], f32, tag="negm")
        eexp = small.tile([P, NT], f32, tag="eexp")
        ssum = small.tile([P, 1], f32, tag="ssum")
        mall = small.tile([P, 1], f32, tag="mall")
        wt = small.tile([P, 1], f32, tag="wt")
        swt = small.tile([P, 1], f32, tag="swt")
        zt = small.tile([P, 1], f32, tag="zt")
        rzt = small.tile([P, 1], f32, tag="rzt")
        attn = small.tile([P, NT], f32, tag="attn")
        outt = small.tile([P, 32], f32, tag="outt")

        nc.vector.reduce_max(out=mst[:], in_=scores[:], axis=mybir.AxisListType.X)
        nc.vector.tensor_scalar_mul(negm[:], mst[:], -1.0)
        nc.scalar.activation(
            out=eexp[:], in_=scores[:], func=Act.Exp,
            bias=negm[:], scale=1.0, accum_out=ssum[:],
        )
        nc.gpsimd.partition_all_reduce(
            mall[:], mst[:], channels=P, reduce_op=bass_isa.ReduceOp.max
        )
        # wt = exp(m_p - M) via func(scale*in + bias) with in=M, scale=-1,
        # bias=m_p — avoids a separate negation on the tail chain.
        nc.scalar.activation(
            out=wt[:], in_=mall[:], func=Act.Exp, bias=mst[:], scale=-1.0
        )
        nc.vector.tensor_mul(swt[:], wt[:], ssum[:])
        nc.gpsimd.partition_all_reduce(
            zt[:], swt[:], channels=P, reduce_op=bass_isa.ReduceOp.add
        )
        nc.vector.reciprocal(rzt[:], zt[:])
        # attn = (eexp * wt) * (1/Z) in one two-scalar op.
        nc.vector.tensor_scalar(
            out=attn[:], in0=eexp[:], scalar1=wt[:], scalar2=rzt[:],
            op0=Alu.mult, op1=Alu.mult,
        )
        # 32x32 block-diagonal transposes so the store has contiguous runs:
        # outt[32j + c, p'] = attn[32j + p', c] = value at l = 128c + 32j + p'
        for j in range(P // 32):
            nc.vector.transpose(
                out=outt[32 * j : 32 * j + 32, :],
                in_=attn[32 * j : 32 * j + 32, :],
            )
        nc.sync.dma_start(out=o_r[b], in_=outt[:])


def _build(reps=1):
    _ensure_concourse()
    import concourse.bacc as bacc
    import concourse.tile as tile
    from concourse import mybir

    nc = bacc.Bacc("TRN2", target_bir_lowering=False, debug=False, num_devices=N_CORES)
    e = nc.dram_tensor("e", [L, B, H], mybir.dt.float32, kind="ExternalInput")
    h = nc.dram_tensor("h", [B, H], mybir.dt.float32, kind="ExternalInput")
    o = nc.dram_tensor("o", [B, L], mybir.dt.float32, kind="ExternalOutput")
    with tile.TileContext(nc) as tc:
        _body(tc, e.ap(), h.ap(), o.ap(), reps=reps)
    nc.compile()
    return nc


def _get_nc(reps=1):
    key = f"nc{reps}"
    if key not in _CACHE:
        _CACHE[key] = _build(reps=reps)
    return _CACHE[key]


def make_in_maps(hidden, encoder_outputs):
    hidden = np.asarray(hidden, dtype=np.float32)
    encoder_outputs = np.asarray(encoder_outputs, dtype=np.float32)
    in_maps = []
    for c in range(N_CORES):
        b0 = c * B
        in_maps.append(
            {
                "e": np.ascontiguousarray(encoder_outputs[:, b0 : b0 + B, :]),
                "h": np.ascontiguousarray(hidden[0, b0 : b0 + B, :]),
            }
        )
    return in_maps


def kernel(hidden, encoder_outputs, **run_kwargs):
    _ensure_concourse()
    from concourse import bass_utils

    nc = _get_nc()
    in_maps = make_in_maps(hidden, encoder_outputs)
    res = bass_utils.run_bass_kernel_spmd(
        nc, in_maps, core_ids=list(range(N_CORES)), **run_kwargs
    )
    out = np.concatenate([res.results[c]["o"] for c in range(N_CORES)], axis=0)
    _CACHE["last_results"] = res
    return out[:, None, :].astype(np.float32)

